# revision 38
# baseline (speedup 1.0000x reference)
"""Trainium2 Bass kernel for DigitConvolutionalModel.

Computation: x[B,784] -> reshape [28,28] -> 3x3 valid conv (single channel)
-> relu -> flatten [676] -> linear to 10 classes.

Strategy (pure data parallel over 8 cores, batch-sharded; per core 4096 rows
are processed in 8 macro-tiles of 512):
  * The conv is expressed as a banded matmul: h[b, o] = sum_p x[b, p] * Wband[p, o]
    where o = 28*oi + oj is the conv output embedded in a 768-slot vector
    (stride-28 embedding; invalid o slots have zero weights).  Wband has
    bandwidth 59, so for 128-wide o-chunks only two 128-row pixel-chunk
    blocks of Wband are nonzero -> 2 matmuls per o-chunk (the second with
    K=16 for the last chunk).  All matmul operands are bf16 (fp32 PSUM
    accumulation), rel err vs the fp32 reference ~4e-3.
  * Input loads are SWDGE DMAs with an inline f32->bf16 cast: one DMA per
    macro-tile for the 768 aligned pixel columns, plus a single up-front DMA
    for all 16-column tails.
  * The tensor engine contracts over the partition dim, so x must be
    transposed to pixel-major (x^T).  Most macro-tiles do this with a single
    DMA-xbar transpose instruction ([128, 3072] bf16 -> 24 transposed
    128x128 chunks written straight to SBUF); every pe_every-th macro uses
    TensorE transpose-mode instead, balancing DMA-fabric vs PE occupancy.
    The 16-pixel tails are always PE-transposed.
  * conv matmuls produce h^T [o-chunk, batch] in PSUM (fp32 accumulate);
    ReLU + cast to bf16 happens in the PSUM->SBUF copy on the Scalar
    engine; the FC layer is 6 accumulating matmuls with the (embedded,
    host-prepared) fc weight as the stationary operand, yielding
    out^T [10, batch] in PSUM.  Bias is added via a per-partition
    tensor_scalar during the PSUM->SBUF copy, then small PE transposes
    bring the result back to batch-major and one DMA per macro-tile
    stores [512, 10] contiguously.
  * After Tile scheduling, a post-pass hoists excess per-instruction
    semaphore waits into standalone EventSemaphore instructions (this
    walrus build only accepts one sync-wait command per instruction).
"""

import sys

for _p in ("/opt/trn_rl_repo",):
    if _p not in sys.path:
        sys.path.insert(0, _p)

import ml_dtypes
import numpy as np

import concourse.bass as bass
import concourse.mybir as mybir
from concourse.bass_utils import run_bass_kernel_spmd
from concourse.tile import TileContext

B = 32768
PIX = 784  # 28*28
EMB = 768  # 6*128; embedded conv-output length (o = 28*oi + oj, max 725)
NCLS = 10
NCORES = 8
BL = B // NCORES  # rows per core
MACRO = 512  # batch rows processed per macro-tile
_BF16 = ml_dtypes.bfloat16


def _host_weights(conv_w, fc_w, fc_b):
    """Build the banded conv matrix and embedded fc weights on the host."""
    oi = np.repeat(np.arange(26), 26)
    oj = np.tile(np.arange(26), 26)
    o = 28 * oi + oj  # embedded output index
    wband = np.zeros((896, EMB), np.float32)
    for ki in range(3):
        for kj in range(3):
            wband[o + 28 * ki + kj, o] = conv_w[ki, kj]
    w1 = np.ascontiguousarray(
        np.stack([wband[128 * q : 128 * (q + 1), 128 * q : 128 * (q + 1)] for q in range(6)])
    )
    w2 = np.ascontiguousarray(
        np.stack([wband[128 * (q + 1) : 128 * (q + 2), 128 * q : 128 * (q + 1)] for q in range(6)])
    )
    fct = np.zeros((EMB, NCLS), np.float32)
    fct[o, :] = fc_w[:, 26 * oi + oj].T
    fcw = np.ascontiguousarray(np.stack([fct[128 * q : 128 * (q + 1)] for q in range(6)]))
    return (
        w1.astype(_BF16),
        w2.astype(_BF16),
        fcw.astype(_BF16),
        np.ascontiguousarray(fc_b.reshape(NCLS, 1)).astype(np.float32),
    )


def _split_waits(nc, max_waits=1):
    """Hoist excess semaphore waits into standalone EventSemaphore
    instructions.  Walrus codegen only supports a small number of sync-wait
    commands per ISA instruction; the Tile scheduler can attach more
    (producer dep + DMA-lane reuse + first-use deps)."""
    for f in nc.m.functions:
        for blk in f.blocks:
            new = []
            changed = False
            for inst in blk.instructions:
                si = inst.sync_info
                if si is not None and len(si.on_wait) > max_waits:
                    waits = list(si.on_wait)
                    excess, keep = waits[:-max_waits], waits[-max_waits:]
                    for i, w in enumerate(excess):
                        new.append(
                            mybir.InstEventSemaphore(
                                name=f"{inst.name}-prewait{i}",
                                engine=inst.engine,
                                ins=[],
                                outs=[],
                                sync_info=mybir.SyncInfo(on_wait=[w], on_update=[]),
                            )
                        )
                    inst.sync_info = mybir.SyncInfo(
                        on_wait=keep, on_update=list(si.on_update)
                    )
                    changed = True
                new.append(inst)
            if changed:
                blk.instructions = new


# Tunables (A/B tested against the CoreSim cost model).
CFG = {
    "load_mode": "swdge_cast",  # or "hwdge_f32"
    "xin_bufs": 4,
    "xt_bufs": 3,
    "ht_bufs": 3,
    "tp_bufs": 4,
    "hp_bufs": 3,
    "op_bufs": 1,
    "batch_store": True,
    "stage": 4,  # ablation: 1=loads 2=+transposes 3=+conv/relu 4=full
    "xbar": True,  # transpose x chunks via DMA xbar instead of TensorE
    "fused": True,  # one load + one xbar transpose per macro
    "pe_every": 3,  # in fused mode, every Nth macro transposes on TensorE
    "load_mult": 1,  # macros per input-load DMA
}


def build_nc(bl=BL, split_waits=True):
    nc = bass.Bass("TRN2")
    x = nc.dram_tensor("x", [bl, PIX], mybir.dt.float32, kind="ExternalInput")
    w1 = nc.dram_tensor("w1", [6, 128, 128], mybir.dt.bfloat16, kind="ExternalInput")
    w2 = nc.dram_tensor("w2", [6, 128, 128], mybir.dt.bfloat16, kind="ExternalInput")
    fcw = nc.dram_tensor("fcw", [6, 128, NCLS], mybir.dt.bfloat16, kind="ExternalInput")
    fcb = nc.dram_tensor("fcb", [NCLS, 1], mybir.dt.float32, kind="ExternalInput")
    eyb = nc.dram_tensor("eyb", [128, 128], mybir.dt.bfloat16, kind="ExternalInput")
    eyf = nc.dram_tensor("eyf", [NCLS, NCLS], mybir.dt.float32, kind="ExternalInput")
    out = nc.dram_tensor("out", [bl, NCLS], mybir.dt.float32, kind="ExternalOutput")

    nm = bl // MACRO
    relu = mybir.ActivationFunctionType.Relu

    with TileContext(nc) as tc:
        with (
            tc.tile_pool(name="const", bufs=1) as cp,
            tc.tile_pool(name="xin", bufs=CFG["xin_bufs"]) as xp,
            tc.tile_pool(name="xt", bufs=CFG["xt_bufs"]) as xtp,
            tc.tile_pool(name="ht", bufs=CFG["ht_bufs"]) as htp,
            tc.tile_pool(name="osb", bufs=3) as osp,
            tc.tile_pool(name="tps", bufs=CFG["tp_bufs"], space="PSUM") as tpp,
            tc.tile_pool(name="hps", bufs=CFG["hp_bufs"], space="PSUM") as hpp,
            tc.tile_pool(name="ops", bufs=CFG["op_bufs"], space="PSUM") as opp,
        ):
            w1s = cp.tile([128, 6 * 128], mybir.dt.bfloat16)
            w2s = cp.tile([128, 6 * 128], mybir.dt.bfloat16)
            fcs = cp.tile([128, 6 * NCLS], mybir.dt.bfloat16)
            fbs = cp.tile([NCLS, 1], mybir.dt.float32)
            eybs = cp.tile([128, 128], mybir.dt.bfloat16)
            eyfs = cp.tile([NCLS, NCLS], mybir.dt.float32)
            for q in range(6):
                nc.sync.dma_start(w1s[:, 128 * q : 128 * (q + 1)], w1[q, :, :])
                nc.sync.dma_start(w2s[:, 128 * q : 128 * (q + 1)], w2[q, :, :])
                nc.sync.dma_start(fcs[:, NCLS * q : NCLS * (q + 1)], fcw[q, :, :])
            nc.sync.dma_start(fbs[:, :], fcb[:, :])
            nc.sync.dma_start(eybs[:, :], eyb[:, :])
            nc.sync.dma_start(eyfs[:, :], eyf[:, :])

            xtl_all = None
            _xfl_cache = [None]
            if CFG["fused"]:
                # All pixel-tail columns (768:784) for the whole shard in one
                # DMA -- per-macro tail loads would each pay the DMA fixed cost.
                xtl_all = cp.tile([128, bl // 128, 16], mybir.dt.bfloat16)
                nc.gpsimd.dma_start(
                    out=xtl_all[:, :, :],
                    in_=x[:, 768:784].rearrange("(g p) c -> p g c", p=128),
                )

            for m in range(nm):
                if CFG["fused"]:
                    nbc = MACRO // 128
                    lm = CFG["load_mult"]
                    if m % lm == 0:
                        # [p, g, col]: batch row 128g+p, pixel col (768 aligned)
                        xfl = xp.tile(
                            [128, lm * nbc, 768], mybir.dt.bfloat16, tag="xf4"
                        )
                        r0 = m * MACRO
                        nc.gpsimd.dma_start(
                            out=xfl[:, :, :],
                            in_=x[r0 : r0 + lm * MACRO, 0:768].rearrange(
                                "(g p) c -> p g c", p=128
                            ),
                        )
                        _xfl_cache[0] = xfl
                    xf4 = _xfl_cache[0][:, (m % lm) * nbc : (m % lm + 1) * nbc, :]
                    if CFG["stage"] < 2:
                        continue
                    # xtf[p, 6g + c, b] = xf4[b, g, 128c + p]; most macros use a
                    # single xbar-transpose DMA, every pe_every-th macro uses
                    # TensorE transposes instead to balance the DMA vs PE load.
                    xtf = xtp.tile([128, nbc * 6, 128], mybir.dt.bfloat16, tag="xtf")
                    if CFG["pe_every"] and m % CFG["pe_every"] == CFG["pe_every"] - 1:
                        for g in range(nbc):
                            for c in range(6):
                                tp = tpp.tile([128, 128], mybir.dt.bfloat16, tag="tp")
                                nc.tensor.transpose(
                                    tp[:, :],
                                    xf4[:, g, 128 * c : 128 * (c + 1)],
                                    eybs[:, :],
                                )
                                dst = xtf[:, 6 * g + c, :]
                                if c % 2 == 0:
                                    nc.vector.tensor_copy(dst, tp[:, :])
                                else:
                                    nc.scalar.copy(dst, tp[:, :])
                    else:
                        nc.sync.dma_start_transpose(xtf[:, :, :], xf4[:, :, :])
                    # conv rhs view: chunk c -> [p, (g, b)]
                    xt = xtf.rearrange("p (g c2) b -> p c2 g b", g=nbc)
                    xt6 = xtp.tile([16, MACRO], mybir.dt.bfloat16, tag="xt6")
                    for g in range(nbc):
                        tp6 = tpp.tile([16, 128], mybir.dt.bfloat16, tag="tp")
                        nc.tensor.transpose(tp6[:, :], xtl_all[:, m * nbc + g, :], eybs[:, :])
                        nc.vector.tensor_copy(xt6[:, g * 128 : (g + 1) * 128], tp6[:, :])
                    bc_iter = []
                else:
                    xt = xtp.tile([128, 6, MACRO], mybir.dt.bfloat16, tag="xt")
                    xt6 = xtp.tile([16, MACRO], mybir.dt.bfloat16, tag="xt6")
                    bc_iter = range(MACRO // 128)
                for bc in bc_iter:
                    row0 = m * MACRO + bc * 128
                    xf = xp.tile([128, PIX], mybir.dt.bfloat16, tag="xf", name=f"xf_{m}_{bc}")
                    if CFG["load_mode"] == "swdge_cast":
                        # SWDGE DMA with inline f32 -> bf16 cast
                        nc.gpsimd.dma_start(out=xf[:, :], in_=x[row0 : row0 + 128, :])
                    else:
                        xf32 = xp.tile(
                            [128, PIX], mybir.dt.float32, tag="xf32", name=f"xf32_{m}_{bc}"
                        )
                        nc.sync.dma_start(out=xf32[:, :], in_=x[row0 : row0 + 128, :])
                        nc.gpsimd.tensor_copy(xf[:, :], xf32[:, :])
                    if CFG["stage"] < 2:
                        continue
                    if CFG["xbar"]:
                        # One xbar-transpose DMA moves all 6 full 128x128
                        # chunks: out[p, c, b] = xf[b, 128c + p].
                        nc.sync.dma_start_transpose(
                            xt[:, :, bc * 128 : (bc + 1) * 128], xf[:, 0:768]
                        )
                    else:
                        for c in range(6):
                            tp = tpp.tile([128, 128], mybir.dt.bfloat16, tag="tp")
                            nc.tensor.transpose(
                                tp[:, :], xf[:, 128 * c : 128 * (c + 1)], eybs[:, :]
                            )
                            dst = xt[:, c, bc * 128 : (bc + 1) * 128]
                            if c % 2 == 0:
                                nc.vector.tensor_copy(dst, tp[:, :])
                            else:
                                nc.scalar.copy(dst, tp[:, :])
                    tp6 = tpp.tile([16, 128], mybir.dt.bfloat16, tag="tp")
                    nc.tensor.transpose(tp6[:, :], xf[:, 768:784], eybs[:, :])
                    nc.vector.tensor_copy(xt6[:, bc * 128 : (bc + 1) * 128], tp6[:, :])

                if CFG["stage"] < 3:
                    continue
                ops = opp.tile([NCLS, MACRO], mybir.dt.float32, tag="ops")
                for q in range(6):
                    hp = hpp.tile([128, MACRO], mybir.dt.float32, tag="hp")
                    nc.tensor.matmul(
                        hp[:, :], w1s[:, 128 * q : 128 * (q + 1)], xt[:, q],
                        start=True, stop=False,
                    )
                    if q < 5:
                        nc.tensor.matmul(
                            hp[:, :], w2s[:, 128 * q : 128 * (q + 1)], xt[:, q + 1],
                            start=False, stop=True,
                        )
                    else:
                        nc.tensor.matmul(
                            hp[:, :], w2s[0:16, 128 * 5 : 128 * 6], xt6[:, :],
                            start=False, stop=True,
                        )
                    ht = htp.tile([128, MACRO], mybir.dt.bfloat16, tag="ht")
                    nc.scalar.activation(ht[:, :], hp[:, :], relu)
                    if CFG["stage"] >= 4:
                        nc.tensor.matmul(
                            ops[:, :], fcs[:, NCLS * q : NCLS * (q + 1)], ht[:, :],
                            start=(q == 0), stop=(q == 5),
                        )

                if CFG["stage"] < 4:
                    continue
                ot = osp.tile([NCLS, MACRO], mybir.dt.float32, tag="ot")
                nc.vector.tensor_scalar_add(ot[:, :], ops[:, :], fbs[:, :])
                if CFG["batch_store"]:
                    nbc = MACRO // 128
                    ob4 = osp.tile([128, nbc * NCLS], mybir.dt.float32, tag="ob4")
                    for bc in range(nbc):
                        op2 = tpp.tile([128, NCLS], mybir.dt.float32, tag="tp")
                        nc.tensor.transpose(
                            op2[:, :], ot[:, bc * 128 : (bc + 1) * 128], eyfs[:, :]
                        )
                        nc.vector.tensor_copy(
                            ob4[:, bc * NCLS : (bc + 1) * NCLS], op2[:, :]
                        )
                    nc.sync.dma_start(
                        out[m * MACRO : (m + 1) * MACRO, :].rearrange(
                            "(b p) c -> p b c", p=128
                        ),
                        ob4.rearrange("p (b c) -> p b c", c=NCLS),
                    )
                else:
                    for bc in range(MACRO // 128):
                        row0 = m * MACRO + bc * 128
                        op2 = tpp.tile([128, NCLS], mybir.dt.float32, tag="tp")
                        nc.tensor.transpose(
                            op2[:, :], ot[:, bc * 128 : (bc + 1) * 128], eyfs[:, :]
                        )
                        ob = osp.tile([128, NCLS], mybir.dt.float32, tag="ob", name=f"ob_{m}_{bc}")
                        nc.vector.tensor_copy(ob[:, :], op2[:, :])
                        nc.sync.dma_start(out[row0 : row0 + 128, :], ob[:, :])
    if split_waits:
        _split_waits(nc)
    return nc


_CACHED = {}


def _get_nc(bl):
    if bl not in _CACHED:
        _CACHED[bl] = build_nc(bl)
    return _CACHED[bl]


def kernel(x, conv_w, fc_w, fc_b):
    x = np.ascontiguousarray(np.asarray(x, dtype=np.float32))
    conv_w = np.asarray(conv_w, dtype=np.float32)
    fc_w = np.asarray(fc_w, dtype=np.float32)
    fc_b = np.asarray(fc_b, dtype=np.float32)

    w1, w2, fcw, fbv = _host_weights(conv_w, fc_w, fc_b)
    eyb = np.eye(128, dtype=np.float32).astype(_BF16)
    eyf = np.eye(NCLS, dtype=np.float32)

    nc = _get_nc(BL)
    in_maps = []
    for c in range(NCORES):
        in_maps.append(
            {
                "x": x[c * BL : (c + 1) * BL],
                "w1": w1,
                "w2": w2,
                "fcw": fcw,
                "fcb": fbv,
                "eyb": eyb,
                "eyf": eyf,
            }
        )
    # The axon-proxied NeuronCores occasionally come up wedged
    # (NRT_EXEC_UNIT_UNRECOVERABLE) on the first execute after idle periods;
    # a retry on a fresh execute reliably recovers.
    last_err = None
    for _attempt in range(3):
        try:
            res = run_bass_kernel_spmd(nc, in_maps, core_ids=list(range(NCORES)))
            break
        except Exception as e:  # noqa: BLE001
            last_err = e
            if "UNRECOVERABLE" not in str(e) and "desynced" not in str(e):
                raise
    else:
        raise last_err
    out = np.concatenate([np.asarray(r["out"]) for r in res.results], axis=0)
    return out


if __name__ == "__main__":
    rng = np.random.default_rng(0)
    xs = rng.standard_normal((B, PIX), dtype=np.float32)
    cw = rng.standard_normal((3, 3), dtype=np.float32)
    fw = (rng.standard_normal((NCLS, 676)) * 0.05).astype(np.float32)
    fb = (rng.standard_normal((NCLS,)) * 0.05).astype(np.float32)
    res = kernel(xs, cw, fw, fb)
    print(res.shape, res.dtype)


# revision 48
# speedup vs baseline: 1.0050x; 1.0050x over previous
"""Trainium2 Bass kernel for DigitConvolutionalModel.

Computation: x[B,784] -> reshape [28,28] -> 3x3 valid conv (single channel)
-> relu -> flatten [676] -> linear to 10 classes.

Strategy (pure data parallel over 8 cores, batch-sharded; per core 4096 rows
are processed in 8 macro-tiles of 512):
  * The conv is expressed as a banded matmul: h[b, o] = sum_p x[b, p] * Wband[p, o]
    where o = 28*oi + oj is the conv output embedded in a 768-slot vector
    (stride-28 embedding; invalid o slots have zero weights).  Wband has
    bandwidth 59, so for 128-wide o-chunks only two 128-row pixel-chunk
    blocks of Wband are nonzero -> 2 matmuls per o-chunk (the second with
    K=16 for the last chunk).  All matmul operands are bf16 (fp32 PSUM
    accumulation), rel err vs the fp32 reference ~4e-3.
  * Input loads are SWDGE DMAs with an inline f32->bf16 cast: one DMA per
    macro-tile for the 768 aligned pixel columns, plus a single up-front DMA
    for all 16-column tails.
  * The tensor engine contracts over the partition dim, so x must be
    transposed to pixel-major (x^T).  Most macro-tiles do this with a single
    DMA-xbar transpose instruction ([128, 3072] bf16 -> 24 transposed
    128x128 chunks written straight to SBUF); every pe_every-th macro uses
    TensorE transpose-mode instead, balancing DMA-fabric vs PE occupancy.
    The 16-pixel tails are always PE-transposed.
  * conv matmuls produce h^T [o-chunk, batch] in PSUM (fp32 accumulate);
    ReLU + cast to bf16 happens in the PSUM->SBUF copy on the Scalar
    engine; the FC layer is 6 accumulating matmuls with the (embedded,
    host-prepared) fc weight as the stationary operand, yielding
    out^T [10, batch] in PSUM.  Bias is added via a per-partition
    tensor_scalar during the PSUM->SBUF copy, then small PE transposes
    bring the result back to batch-major and one DMA per macro-tile
    stores [512, 10] contiguously.
  * After Tile scheduling, a post-pass hoists excess per-instruction
    semaphore waits into standalone EventSemaphore instructions (this
    walrus build only accepts one sync-wait command per instruction).
"""

import sys

for _p in ("/opt/trn_rl_repo",):
    if _p not in sys.path:
        sys.path.insert(0, _p)

import ml_dtypes
import numpy as np

import concourse.bass as bass
import concourse.mybir as mybir
from concourse.bass_utils import run_bass_kernel_spmd
from concourse.tile import TileContext

B = 32768
PIX = 784  # 28*28
EMB = 768  # 6*128; embedded conv-output length (o = 28*oi + oj, max 725)
NCLS = 10
NCORES = 8
BL = B // NCORES  # rows per core
MACRO = 512  # batch rows processed per macro-tile
_BF16 = ml_dtypes.bfloat16


def _host_weights(conv_w, fc_w, fc_b):
    """Build the banded conv matrix and embedded fc weights on the host."""
    oi = np.repeat(np.arange(26), 26)
    oj = np.tile(np.arange(26), 26)
    o = 28 * oi + oj  # embedded output index
    wband = np.zeros((896, EMB), np.float32)
    for ki in range(3):
        for kj in range(3):
            wband[o + 28 * ki + kj, o] = conv_w[ki, kj]
    w1 = np.ascontiguousarray(
        np.stack([wband[128 * q : 128 * (q + 1), 128 * q : 128 * (q + 1)] for q in range(6)])
    )
    w2 = np.ascontiguousarray(
        np.stack([wband[128 * (q + 1) : 128 * (q + 2), 128 * q : 128 * (q + 1)] for q in range(6)])
    )
    fct = np.zeros((EMB, NCLS), np.float32)
    fct[o, :] = fc_w[:, 26 * oi + oj].T
    fcw = np.ascontiguousarray(np.stack([fct[128 * q : 128 * (q + 1)] for q in range(6)]))
    return (
        w1.astype(_BF16),
        w2.astype(_BF16),
        fcw.astype(_BF16),
        np.ascontiguousarray(fc_b.reshape(NCLS, 1)).astype(np.float32),
    )


def _host_packs(conv_w, fc_w, fc_b):
    """Pack all constants into two arrays so they load in two DMAs.

    wpack [128, 1724] bf16 = w1 (6x128 cols) | w2 (6x128) | fcw (6x10) | eye128
    fpack [10, 11] f32    = fc_b | eye10
    """
    w1, w2, fcw, fbv = _host_weights(conv_w, fc_w, fc_b)
    cols = [w1[q] for q in range(6)] + [w2[q] for q in range(6)] + [fcw[q] for q in range(6)]
    cols.append(np.eye(128, dtype=np.float32).astype(_BF16))
    wpack = np.ascontiguousarray(np.concatenate(cols, axis=1))
    fpack = np.ascontiguousarray(
        np.concatenate([fbv, np.eye(NCLS, dtype=np.float32)], axis=1)
    ).astype(np.float32)
    return wpack, fpack


def _split_waits(nc, max_waits=1):
    """Hoist excess semaphore waits into standalone EventSemaphore
    instructions.  Walrus codegen only supports a small number of sync-wait
    commands per ISA instruction; the Tile scheduler can attach more
    (producer dep + DMA-lane reuse + first-use deps)."""
    for f in nc.m.functions:
        for blk in f.blocks:
            new = []
            changed = False
            for inst in blk.instructions:
                si = inst.sync_info
                if si is not None and len(si.on_wait) > max_waits:
                    waits = list(si.on_wait)
                    excess, keep = waits[:-max_waits], waits[-max_waits:]
                    for i, w in enumerate(excess):
                        new.append(
                            mybir.InstEventSemaphore(
                                name=f"{inst.name}-prewait{i}",
                                engine=inst.engine,
                                ins=[],
                                outs=[],
                                sync_info=mybir.SyncInfo(on_wait=[w], on_update=[]),
                            )
                        )
                    inst.sync_info = mybir.SyncInfo(
                        on_wait=keep, on_update=list(si.on_update)
                    )
                    changed = True
                new.append(inst)
            if changed:
                blk.instructions = new


# Tunables (A/B tested against the CoreSim cost model).
CFG = {
    "load_mode": "swdge_cast",  # or "hwdge_f32"
    "xin_bufs": 4,
    "xt_bufs": 3,
    "ht_bufs": 3,
    "tp_bufs": 4,
    "hp_bufs": 3,
    "op_bufs": 1,
    "batch_store": True,
    "stage": 4,  # ablation: 1=loads 2=+transposes 3=+conv/relu 4=full
    "xbar": True,  # transpose x chunks via DMA xbar instead of TensorE
    "fused": True,  # one load + one xbar transpose per macro
    "pe_every": 3,  # in fused mode, every Nth macro transposes on TensorE
    "pe_set": [0],  # explicit macro indices using TensorE transposes (overrides pe_every)
    "load_mult": 1,  # macros per input-load DMA
    "relu_dve": 0,  # how many of the 6 per-macro relus run on DVE (rest ACT)
}


def build_nc(bl=BL, split_waits=True):
    # Default SWDGE descriptor ring (16 KiB = 1024 descs) only holds ~2 input
    # loads' descriptors, serializing descriptor-gen behind transfers.
    nc = bass.Bass("TRN2", dynamic_dma_scratch_size=CFG.get("dma_scratch", 65536))
    x = nc.dram_tensor("x", [bl, PIX], mybir.dt.float32, kind="ExternalInput")
    wpk = nc.dram_tensor("wpack", [128, 1724], mybir.dt.bfloat16, kind="ExternalInput")
    fpk = nc.dram_tensor("fpack", [NCLS, 11], mybir.dt.float32, kind="ExternalInput")
    out = nc.dram_tensor("out", [bl, NCLS], mybir.dt.float32, kind="ExternalOutput")

    nm = bl // MACRO
    relu = mybir.ActivationFunctionType.Relu

    with TileContext(nc) as tc:
        with (
            tc.tile_pool(name="const", bufs=1) as cp,
            tc.tile_pool(name="xin", bufs=CFG["xin_bufs"]) as xp,
            tc.tile_pool(name="xt", bufs=CFG["xt_bufs"]) as xtp,
            tc.tile_pool(name="ht", bufs=CFG["ht_bufs"]) as htp,
            tc.tile_pool(name="osb", bufs=3) as osp,
            tc.tile_pool(name="tps", bufs=CFG["tp_bufs"], space="PSUM") as tpp,
            tc.tile_pool(name="hps", bufs=CFG["hp_bufs"], space="PSUM") as hpp,
            tc.tile_pool(name="ops", bufs=CFG["op_bufs"], space="PSUM") as opp,
        ):
            nbc = MACRO // 128
            _xfl_cache = [None]
            # Emit the first macro's input load BEFORE the constants so the
            # DMA engines start streaming x immediately (PE startup latency).
            if CFG["fused"]:
                xfl0 = xp.tile([128, nbc, 768], mybir.dt.bfloat16, tag="xf4", name="xf4_first")
                nc.gpsimd.dma_start(
                    out=xfl0[:, :, :],
                    in_=x[0:MACRO, 0:768].rearrange("(g p) c -> p g c", p=128),
                )
                _xfl_cache[0] = xfl0

            wps = cp.tile([128, 1724], mybir.dt.bfloat16)
            nc.sync.dma_start(wps[:, :], wpk[:, :])
            fps = cp.tile([NCLS, 11], mybir.dt.float32)
            nc.sync.dma_start(fps[:, :], fpk[:, :])
            w1s = wps[:, 0:768]
            w2s = wps[:, 768:1536]
            fcs = wps[:, 1536:1596]
            eybs = wps[:, 1596:1724]
            fbs = fps[:, 0:1]
            eyfs = fps[:, 1:11]

            xtl_all = None
            if CFG["fused"]:
                # All pixel-tail columns (768:784) for the whole shard in one
                # DMA -- per-macro tail loads would each pay the DMA fixed cost.
                xtl_all = cp.tile([128, bl // 128, 16], mybir.dt.bfloat16)
                nc.gpsimd.dma_start(
                    out=xtl_all[:, :, :],
                    in_=x[:, 768:784].rearrange("(g p) c -> p g c", p=128),
                )

            for m in range(nm):
                if CFG["fused"]:
                    lm = CFG["load_mult"]
                    if m % lm == 0 and m > 0:
                        # [p, g, col]: batch row 128g+p, pixel col (768 aligned)
                        xfl = xp.tile(
                            [128, lm * nbc, 768], mybir.dt.bfloat16, tag="xf4"
                        )
                        r0 = m * MACRO
                        nc.gpsimd.dma_start(
                            out=xfl[:, :, :],
                            in_=x[r0 : r0 + lm * MACRO, 0:768].rearrange(
                                "(g p) c -> p g c", p=128
                            ),
                        )
                        _xfl_cache[0] = xfl
                    xf4 = _xfl_cache[0][:, (m % lm) * nbc : (m % lm + 1) * nbc, :]
                    if CFG["stage"] < 2:
                        continue
                    # xtf[p, 6g + c, b] = xf4[b, g, 128c + p]; most macros use a
                    # single xbar-transpose DMA, every pe_every-th macro uses
                    # TensorE transposes instead to balance the DMA vs PE load.
                    xtf = xtp.tile([128, nbc * 6, 128], mybir.dt.bfloat16, tag="xtf")
                    if CFG.get("pe_set") is not None:
                        use_pe = m in CFG["pe_set"]
                    else:
                        use_pe = CFG["pe_every"] and m % CFG["pe_every"] == 0
                    if use_pe:
                        for g in range(nbc):
                            for c in range(6):
                                tp = tpp.tile([128, 128], mybir.dt.bfloat16, tag="tp")
                                nc.tensor.transpose(
                                    tp[:, :],
                                    xf4[:, g, 128 * c : 128 * (c + 1)],
                                    eybs[:, :],
                                )
                                dst = xtf[:, 6 * g + c, :]
                                if c % 2 == 0:
                                    nc.vector.tensor_copy(dst, tp[:, :])
                                else:
                                    nc.scalar.copy(dst, tp[:, :])
                    else:
                        nc.sync.dma_start_transpose(xtf[:, :, :], xf4[:, :, :])
                    # conv rhs view: chunk c -> [p, (g, b)]
                    xt = xtf.rearrange("p (g c2) b -> p c2 g b", g=nbc)
                    xt6 = xtp.tile([16, MACRO], mybir.dt.bfloat16, tag="xt6")
                    for g in range(nbc):
                        tp6 = tpp.tile([16, 128], mybir.dt.bfloat16, tag="tp")
                        nc.tensor.transpose(tp6[:, :], xtl_all[:, m * nbc + g, :], eybs[:, :])
                        dst6 = xt6[:, g * 128 : (g + 1) * 128]
                        if g % 2 == 0:
                            nc.scalar.copy(dst6, tp6[:, :])
                        else:
                            nc.vector.tensor_copy(dst6, tp6[:, :])
                    bc_iter = []
                else:
                    xt = xtp.tile([128, 6, MACRO], mybir.dt.bfloat16, tag="xt")
                    xt6 = xtp.tile([16, MACRO], mybir.dt.bfloat16, tag="xt6")
                    bc_iter = range(MACRO // 128)
                for bc in bc_iter:
                    row0 = m * MACRO + bc * 128
                    xf = xp.tile([128, PIX], mybir.dt.bfloat16, tag="xf", name=f"xf_{m}_{bc}")
                    if CFG["load_mode"] == "swdge_cast":
                        # SWDGE DMA with inline f32 -> bf16 cast
                        nc.gpsimd.dma_start(out=xf[:, :], in_=x[row0 : row0 + 128, :])
                    else:
                        xf32 = xp.tile(
                            [128, PIX], mybir.dt.float32, tag="xf32", name=f"xf32_{m}_{bc}"
                        )
                        nc.sync.dma_start(out=xf32[:, :], in_=x[row0 : row0 + 128, :])
                        nc.gpsimd.tensor_copy(xf[:, :], xf32[:, :])
                    if CFG["stage"] < 2:
                        continue
                    if CFG["xbar"]:
                        # One xbar-transpose DMA moves all 6 full 128x128
                        # chunks: out[p, c, b] = xf[b, 128c + p].
                        nc.sync.dma_start_transpose(
                            xt[:, :, bc * 128 : (bc + 1) * 128], xf[:, 0:768]
                        )
                    else:
                        for c in range(6):
                            tp = tpp.tile([128, 128], mybir.dt.bfloat16, tag="tp")
                            nc.tensor.transpose(
                                tp[:, :], xf[:, 128 * c : 128 * (c + 1)], eybs[:, :]
                            )
                            dst = xt[:, c, bc * 128 : (bc + 1) * 128]
                            if c % 2 == 0:
                                nc.vector.tensor_copy(dst, tp[:, :])
                            else:
                                nc.scalar.copy(dst, tp[:, :])
                    tp6 = tpp.tile([16, 128], mybir.dt.bfloat16, tag="tp")
                    nc.tensor.transpose(tp6[:, :], xf[:, 768:784], eybs[:, :])
                    nc.vector.tensor_copy(xt6[:, bc * 128 : (bc + 1) * 128], tp6[:, :])

                if CFG["stage"] < 3:
                    continue
                ops = opp.tile([NCLS, MACRO], mybir.dt.float32, tag="ops")
                for q in range(6):
                    hp = hpp.tile([128, MACRO], mybir.dt.float32, tag="hp")
                    nc.tensor.matmul(
                        hp[:, :], w1s[:, 128 * q : 128 * (q + 1)], xt[:, q],
                        start=True, stop=False,
                    )
                    if q < 5:
                        nc.tensor.matmul(
                            hp[:, :], w2s[:, 128 * q : 128 * (q + 1)], xt[:, q + 1],
                            start=False, stop=True,
                        )
                    else:
                        nc.tensor.matmul(
                            hp[:, :], w2s[0:16, 128 * 5 : 128 * 6], xt6[:, :],
                            start=False, stop=True,
                        )
                    ht = htp.tile([128, MACRO], mybir.dt.bfloat16, tag="ht")
                    # ReLU during PSUM->SBUF drain; split between the Scalar
                    # and Vector engines to balance their load.
                    if q < CFG["relu_dve"]:
                        nc.vector.tensor_scalar_max(ht[:, :], hp[:, :], 0.0)
                    else:
                        nc.scalar.activation(ht[:, :], hp[:, :], relu)
                    if CFG["stage"] >= 4:
                        nc.tensor.matmul(
                            ops[:, :], fcs[:, NCLS * q : NCLS * (q + 1)], ht[:, :],
                            start=(q == 0), stop=(q == 5),
                        )

                if CFG["stage"] < 4:
                    continue
                ot = osp.tile([NCLS, MACRO], mybir.dt.float32, tag="ot")
                nc.vector.tensor_scalar_add(ot[:, :], ops[:, :], fbs[:, :])
                if CFG["batch_store"]:
                    nbc = MACRO // 128
                    ob4 = osp.tile([128, nbc * NCLS], mybir.dt.float32, tag="ob4")
                    for bc in range(nbc):
                        op2 = tpp.tile([128, NCLS], mybir.dt.float32, tag="tp")
                        nc.tensor.transpose(
                            op2[:, :], ot[:, bc * 128 : (bc + 1) * 128], eyfs[:, :]
                        )
                        nc.vector.tensor_copy(
                            ob4[:, bc * NCLS : (bc + 1) * NCLS], op2[:, :]
                        )
                    nc.sync.dma_start(
                        out[m * MACRO : (m + 1) * MACRO, :].rearrange(
                            "(b p) c -> p b c", p=128
                        ),
                        ob4.rearrange("p (b c) -> p b c", c=NCLS),
                    )
                else:
                    for bc in range(MACRO // 128):
                        row0 = m * MACRO + bc * 128
                        op2 = tpp.tile([128, NCLS], mybir.dt.float32, tag="tp")
                        nc.tensor.transpose(
                            op2[:, :], ot[:, bc * 128 : (bc + 1) * 128], eyfs[:, :]
                        )
                        ob = osp.tile([128, NCLS], mybir.dt.float32, tag="ob", name=f"ob_{m}_{bc}")
                        nc.vector.tensor_copy(ob[:, :], op2[:, :])
                        nc.sync.dma_start(out[row0 : row0 + 128, :], ob[:, :])
    if split_waits:
        _split_waits(nc)
    return nc


_CACHED = {}


def _get_nc(bl):
    if bl not in _CACHED:
        _CACHED[bl] = build_nc(bl)
    return _CACHED[bl]


def kernel(x, conv_w, fc_w, fc_b):
    x = np.ascontiguousarray(np.asarray(x, dtype=np.float32))
    conv_w = np.asarray(conv_w, dtype=np.float32)
    fc_w = np.asarray(fc_w, dtype=np.float32)
    fc_b = np.asarray(fc_b, dtype=np.float32)

    wpack, fpack = _host_packs(conv_w, fc_w, fc_b)

    nc = _get_nc(BL)
    in_maps = []
    for c in range(NCORES):
        in_maps.append(
            {
                "x": x[c * BL : (c + 1) * BL],
                "wpack": wpack,
                "fpack": fpack,
            }
        )
    # The axon-proxied NeuronCores occasionally come up wedged
    # (NRT_EXEC_UNIT_UNRECOVERABLE) on the first execute after idle periods;
    # a retry on a fresh execute reliably recovers.
    last_err = None
    for _attempt in range(3):
        try:
            res = run_bass_kernel_spmd(nc, in_maps, core_ids=list(range(NCORES)))
            break
        except Exception as e:  # noqa: BLE001
            last_err = e
            if "UNRECOVERABLE" not in str(e) and "desynced" not in str(e):
                raise
    else:
        raise last_err
    out = np.concatenate([np.asarray(r["out"]) for r in res.results], axis=0)
    return out


if __name__ == "__main__":
    rng = np.random.default_rng(0)
    xs = rng.standard_normal((B, PIX), dtype=np.float32)
    cw = rng.standard_normal((3, 3), dtype=np.float32)
    fw = (rng.standard_normal((NCLS, 676)) * 0.05).astype(np.float32)
    fb = (rng.standard_normal((NCLS,)) * 0.05).astype(np.float32)
    res = kernel(xs, cw, fw, fb)
    print(res.shape, res.dtype)


# revision 54
# speedup vs baseline: 1.0761x; 1.0707x over previous
"""Trainium2 Bass kernel for DigitConvolutionalModel.

Computation: x[B,784] -> reshape [28,28] -> 3x3 valid conv (single channel)
-> relu -> flatten [676] -> linear to 10 classes.

Strategy (pure data parallel over 8 cores, batch-sharded; per core 4096 rows
are processed in 8 macro-tiles of 512):
  * The conv is expressed as a banded matmul: h[b, o] = sum_p x[b, p] * Wband[p, o]
    where o = 28*oi + oj is the conv output embedded in a 768-slot vector
    (stride-28 embedding; invalid o slots have zero weights).  Wband has
    bandwidth 59, so for 128-wide o-chunks only two 128-row pixel-chunk
    blocks of Wband are nonzero -> 2 matmuls per o-chunk (the second with
    K=16 for the last chunk).  All matmul operands are bf16 (fp32 PSUM
    accumulation), rel err vs the fp32 reference ~4e-3.
  * Input loads are SWDGE DMAs with an inline f32->bf16 cast: one DMA per
    macro-tile for the 768 aligned pixel columns, plus a single up-front DMA
    for all 16-column tails.
  * The tensor engine contracts over the partition dim, so x must be
    transposed to pixel-major (x^T).  Most macro-tiles do this with a single
    DMA-xbar transpose instruction ([128, 3072] bf16 -> 24 transposed
    128x128 chunks written straight to SBUF); every pe_every-th macro uses
    TensorE transpose-mode instead, balancing DMA-fabric vs PE occupancy.
    The 16-pixel tails are always PE-transposed.
  * conv matmuls produce h^T [o-chunk, batch] in PSUM (fp32 accumulate);
    ReLU + cast to bf16 happens in the PSUM->SBUF copy on the Scalar
    engine; the FC layer is 6 accumulating matmuls with the (embedded,
    host-prepared) fc weight as the stationary operand, yielding
    out^T [10, batch] in PSUM.  Bias is added via a per-partition
    tensor_scalar during the PSUM->SBUF copy, then small PE transposes
    bring the result back to batch-major and one DMA per macro-tile
    stores [512, 10] contiguously.
  * After Tile scheduling, a post-pass hoists excess per-instruction
    semaphore waits into standalone EventSemaphore instructions (this
    walrus build only accepts one sync-wait command per instruction).
"""

import sys

for _p in ("/opt/trn_rl_repo",):
    if _p not in sys.path:
        sys.path.insert(0, _p)

import ml_dtypes
import numpy as np

import concourse.bass as bass
import concourse.mybir as mybir
from concourse.bass_utils import run_bass_kernel_spmd
from concourse.tile import TileContext

B = 32768
PIX = 784  # 28*28
EMB = 768  # 6*128; embedded conv-output length (o = 28*oi + oj, max 725)
NCLS = 10
NCORES = 8
BL = B // NCORES  # rows per core
MACRO = 512  # batch rows processed per macro-tile
_BF16 = ml_dtypes.bfloat16


def _host_weights(conv_w, fc_w, fc_b):
    """Build the banded conv matrix and embedded fc weights on the host."""
    oi = np.repeat(np.arange(26), 26)
    oj = np.tile(np.arange(26), 26)
    o = 28 * oi + oj  # embedded output index
    wband = np.zeros((896, EMB), np.float32)
    for ki in range(3):
        for kj in range(3):
            wband[o + 28 * ki + kj, o] = conv_w[ki, kj]
    w1 = np.ascontiguousarray(
        np.stack([wband[128 * q : 128 * (q + 1), 128 * q : 128 * (q + 1)] for q in range(6)])
    )
    w2 = np.ascontiguousarray(
        np.stack([wband[128 * (q + 1) : 128 * (q + 2), 128 * q : 128 * (q + 1)] for q in range(6)])
    )
    fct = np.zeros((EMB, NCLS), np.float32)
    fct[o, :] = fc_w[:, 26 * oi + oj].T
    fcw = np.ascontiguousarray(np.stack([fct[128 * q : 128 * (q + 1)] for q in range(6)]))
    return (
        w1.astype(_BF16),
        w2.astype(_BF16),
        fcw.astype(_BF16),
        np.ascontiguousarray(fc_b.reshape(NCLS, 1)).astype(np.float32),
    )


def _host_packs(conv_w, fc_w, fc_b):
    """Pack all constants into two arrays so they load in two DMAs.

    wpack [128, 1724] bf16 = w1 (6x128 cols) | w2 (6x128) | fcw (6x10) | eye128
    fpack [10, 11] f32    = fc_b | eye10
    """
    w1, w2, fcw, fbv = _host_weights(conv_w, fc_w, fc_b)
    cols = [w1[q] for q in range(6)] + [w2[q] for q in range(6)] + [fcw[q] for q in range(6)]
    cols.append(np.eye(128, dtype=np.float32).astype(_BF16))
    wpack = np.ascontiguousarray(np.concatenate(cols, axis=1))
    fpack = np.ascontiguousarray(
        np.concatenate([fbv, np.eye(NCLS, dtype=np.float32)], axis=1)
    ).astype(np.float32)
    return wpack, fpack


def _split_waits(nc, max_waits=1):
    """Hoist excess semaphore waits into standalone EventSemaphore
    instructions.  Walrus codegen only supports a small number of sync-wait
    commands per ISA instruction; the Tile scheduler can attach more
    (producer dep + DMA-lane reuse + first-use deps)."""
    for f in nc.m.functions:
        for blk in f.blocks:
            new = []
            changed = False
            for inst in blk.instructions:
                si = inst.sync_info
                if si is not None and len(si.on_wait) > max_waits:
                    waits = list(si.on_wait)
                    excess, keep = waits[:-max_waits], waits[-max_waits:]
                    for i, w in enumerate(excess):
                        new.append(
                            mybir.InstEventSemaphore(
                                name=f"{inst.name}-prewait{i}",
                                engine=inst.engine,
                                ins=[],
                                outs=[],
                                sync_info=mybir.SyncInfo(on_wait=[w], on_update=[]),
                            )
                        )
                    inst.sync_info = mybir.SyncInfo(
                        on_wait=keep, on_update=list(si.on_update)
                    )
                    changed = True
                new.append(inst)
            if changed:
                blk.instructions = new


# Tunables (A/B tested against the CoreSim cost model).
CFG = {
    "load_mode": "swdge_cast",  # or "hwdge_f32"
    "xin_bufs": 4,
    "xt_bufs": 3,
    "ht_bufs": 3,
    "tp_bufs": 4,
    "hp_bufs": 3,
    "op_bufs": 1,
    "batch_store": True,
    "stage": 4,  # ablation: 1=loads 2=+transposes 3=+conv/relu 4=full
    "xbar": True,  # transpose x chunks via DMA xbar instead of TensorE
    "fused": True,  # one load + one xbar transpose per macro
    "pe_every": 3,  # in fused mode, every Nth macro transposes on TensorE
    "pe_set": [0],  # explicit macro indices using TensorE transposes (overrides pe_every)
    "load_mult": 1,  # macros per input-load DMA
    "relu_dve": 0,  # how many of the 6 per-macro relus run on DVE (rest ACT)
    "prefetch": 3,  # macro loads emitted before the constants at kernel start
}


def build_nc(bl=BL, split_waits=True):
    # Default SWDGE descriptor ring (16 KiB = 1024 descs) only holds ~2 input
    # loads' descriptors, serializing descriptor-gen behind transfers.
    nc = bass.Bass("TRN2", dynamic_dma_scratch_size=CFG.get("dma_scratch", 65536))
    x = nc.dram_tensor("x", [bl, PIX], mybir.dt.float32, kind="ExternalInput")
    wpk = nc.dram_tensor("wpack", [128, 1724], mybir.dt.bfloat16, kind="ExternalInput")
    fpk = nc.dram_tensor("fpack", [NCLS, 11], mybir.dt.float32, kind="ExternalInput")
    out = nc.dram_tensor("out", [bl, NCLS], mybir.dt.float32, kind="ExternalOutput")

    nm = bl // MACRO
    relu = mybir.ActivationFunctionType.Relu

    with TileContext(nc) as tc:
        with (
            tc.tile_pool(name="const", bufs=1) as cp,
            tc.tile_pool(name="xin", bufs=CFG["xin_bufs"]) as xp,
            tc.tile_pool(name="xt", bufs=CFG["xt_bufs"]) as xtp,
            tc.tile_pool(name="ht", bufs=CFG["ht_bufs"]) as htp,
            tc.tile_pool(name="osb", bufs=3) as osp,
            tc.tile_pool(name="tps", bufs=CFG["tp_bufs"], space="PSUM") as tpp,
            tc.tile_pool(name="hps", bufs=CFG["hp_bufs"], space="PSUM") as hpp,
            tc.tile_pool(name="ops", bufs=CFG["op_bufs"], space="PSUM") as opp,
        ):
            nbc = MACRO // 128
            _xfl_cache = [None]
            _prefetched = {}
            # Emit the first macro loads BEFORE the constants so the DMA
            # engines start streaming x immediately (PE startup latency), and
            # so the serialized load->xbar chain starts with a full pipeline.
            if CFG["fused"]:
                for pm in range(min(CFG.get("prefetch", 1), bl // MACRO)):
                    xflp = xp.tile(
                        [128, nbc, 768], mybir.dt.bfloat16, tag="xf4", name=f"xf4_pre{pm}"
                    )
                    nc.gpsimd.dma_start(
                        out=xflp[:, :, :],
                        in_=x[pm * MACRO : (pm + 1) * MACRO, 0:768].rearrange(
                            "(g p) c -> p g c", p=128
                        ),
                    )
                    _prefetched[pm] = xflp

            wps = cp.tile([128, 1724], mybir.dt.bfloat16)
            nc.sync.dma_start(wps[:, :], wpk[:, :])
            fps = cp.tile([NCLS, 11], mybir.dt.float32)
            nc.sync.dma_start(fps[:, :], fpk[:, :])
            w1s = wps[:, 0:768]
            w2s = wps[:, 768:1536]
            fcs = wps[:, 1536:1596]
            eybs = wps[:, 1596:1724]
            fbs = fps[:, 0:1]
            eyfs = fps[:, 1:11]

            xtl_all = None
            if CFG["fused"]:
                # All pixel-tail columns (768:784) for the whole shard in one
                # DMA -- per-macro tail loads would each pay the DMA fixed cost.
                xtl_all = cp.tile([128, bl // 128, 16], mybir.dt.bfloat16)
                nc.gpsimd.dma_start(
                    out=xtl_all[:, :, :],
                    in_=x[:, 768:784].rearrange("(g p) c -> p g c", p=128),
                )

            for m in range(nm):
                if CFG["fused"]:
                    lm = CFG["load_mult"]
                    if m in _prefetched:
                        _xfl_cache[0] = _prefetched[m]
                    elif m % lm == 0:
                        # [p, g, col]: batch row 128g+p, pixel col (768 aligned)
                        xfl = xp.tile(
                            [128, lm * nbc, 768], mybir.dt.bfloat16, tag="xf4"
                        )
                        r0 = m * MACRO
                        ld = nc.gpsimd.dma_start(
                            out=xfl[:, :, :],
                            in_=x[r0 : r0 + lm * MACRO, 0:768].rearrange(
                                "(g p) c -> p g c", p=128
                            ),
                        )
                        if CFG.get("dma_prio"):
                            ld.ins.bass_priority = -10000 + m
                        _xfl_cache[0] = xfl
                    xf4 = _xfl_cache[0][:, (m % lm) * nbc : (m % lm + 1) * nbc, :]
                    if CFG["stage"] < 2:
                        continue
                    # xtf[p, 6g + c, b] = xf4[b, g, 128c + p]; most macros use a
                    # single xbar-transpose DMA, every pe_every-th macro uses
                    # TensorE transposes instead to balance the DMA vs PE load.
                    xtf = xtp.tile([128, nbc * 6, 128], mybir.dt.bfloat16, tag="xtf")
                    if CFG.get("pe_set") is not None:
                        use_pe = m in CFG["pe_set"]
                    else:
                        use_pe = CFG["pe_every"] and m % CFG["pe_every"] == 0
                    if use_pe:
                        for g in range(nbc):
                            for c in range(6):
                                tp = tpp.tile([128, 128], mybir.dt.bfloat16, tag="tp")
                                nc.tensor.transpose(
                                    tp[:, :],
                                    xf4[:, g, 128 * c : 128 * (c + 1)],
                                    eybs[:, :],
                                )
                                dst = xtf[:, 6 * g + c, :]
                                if c % 2 == 0:
                                    nc.vector.tensor_copy(dst, tp[:, :])
                                else:
                                    nc.scalar.copy(dst, tp[:, :])
                    else:
                        xb = nc.sync.dma_start_transpose(xtf[:, :, :], xf4[:, :, :])
                        if CFG.get("dma_prio"):
                            xb.ins.bass_priority = -9000 + m
                    # conv rhs view: chunk c -> [p, (g, b)]
                    xt = xtf.rearrange("p (g c2) b -> p c2 g b", g=nbc)
                    xt6 = xtp.tile([16, MACRO], mybir.dt.bfloat16, tag="xt6")
                    for g in range(nbc):
                        tp6 = tpp.tile([16, 128], mybir.dt.bfloat16, tag="tp")
                        nc.tensor.transpose(tp6[:, :], xtl_all[:, m * nbc + g, :], eybs[:, :])
                        dst6 = xt6[:, g * 128 : (g + 1) * 128]
                        if g % 2 == 0:
                            nc.scalar.copy(dst6, tp6[:, :])
                        else:
                            nc.vector.tensor_copy(dst6, tp6[:, :])
                    bc_iter = []
                else:
                    xt = xtp.tile([128, 6, MACRO], mybir.dt.bfloat16, tag="xt")
                    xt6 = xtp.tile([16, MACRO], mybir.dt.bfloat16, tag="xt6")
                    bc_iter = range(MACRO // 128)
                for bc in bc_iter:
                    row0 = m * MACRO + bc * 128
                    xf = xp.tile([128, PIX], mybir.dt.bfloat16, tag="xf", name=f"xf_{m}_{bc}")
                    if CFG["load_mode"] == "swdge_cast":
                        # SWDGE DMA with inline f32 -> bf16 cast
                        nc.gpsimd.dma_start(out=xf[:, :], in_=x[row0 : row0 + 128, :])
                    else:
                        xf32 = xp.tile(
                            [128, PIX], mybir.dt.float32, tag="xf32", name=f"xf32_{m}_{bc}"
                        )
                        nc.sync.dma_start(out=xf32[:, :], in_=x[row0 : row0 + 128, :])
                        nc.gpsimd.tensor_copy(xf[:, :], xf32[:, :])
                    if CFG["stage"] < 2:
                        continue
                    if CFG["xbar"]:
                        # One xbar-transpose DMA moves all 6 full 128x128
                        # chunks: out[p, c, b] = xf[b, 128c + p].
                        nc.sync.dma_start_transpose(
                            xt[:, :, bc * 128 : (bc + 1) * 128], xf[:, 0:768]
                        )
                    else:
                        for c in range(6):
                            tp = tpp.tile([128, 128], mybir.dt.bfloat16, tag="tp")
                            nc.tensor.transpose(
                                tp[:, :], xf[:, 128 * c : 128 * (c + 1)], eybs[:, :]
                            )
                            dst = xt[:, c, bc * 128 : (bc + 1) * 128]
                            if c % 2 == 0:
                                nc.vector.tensor_copy(dst, tp[:, :])
                            else:
                                nc.scalar.copy(dst, tp[:, :])
                    tp6 = tpp.tile([16, 128], mybir.dt.bfloat16, tag="tp")
                    nc.tensor.transpose(tp6[:, :], xf[:, 768:784], eybs[:, :])
                    nc.vector.tensor_copy(xt6[:, bc * 128 : (bc + 1) * 128], tp6[:, :])

                if CFG["stage"] < 3:
                    continue
                ops = opp.tile([NCLS, MACRO], mybir.dt.float32, tag="ops")
                for q in range(6):
                    hp = hpp.tile([128, MACRO], mybir.dt.float32, tag="hp")
                    nc.tensor.matmul(
                        hp[:, :], w1s[:, 128 * q : 128 * (q + 1)], xt[:, q],
                        start=True, stop=False,
                    )
                    if q < 5:
                        nc.tensor.matmul(
                            hp[:, :], w2s[:, 128 * q : 128 * (q + 1)], xt[:, q + 1],
                            start=False, stop=True,
                        )
                    else:
                        nc.tensor.matmul(
                            hp[:, :], w2s[0:16, 128 * 5 : 128 * 6], xt6[:, :],
                            start=False, stop=True,
                        )
                    ht = htp.tile([128, MACRO], mybir.dt.bfloat16, tag="ht")
                    # ReLU during PSUM->SBUF drain; split between the Scalar
                    # and Vector engines to balance their load.
                    if q < CFG["relu_dve"]:
                        nc.vector.tensor_scalar_max(ht[:, :], hp[:, :], 0.0)
                    else:
                        nc.scalar.activation(ht[:, :], hp[:, :], relu)
                    if CFG["stage"] >= 4:
                        nc.tensor.matmul(
                            ops[:, :], fcs[:, NCLS * q : NCLS * (q + 1)], ht[:, :],
                            start=(q == 0), stop=(q == 5),
                        )

                if CFG["stage"] < 4:
                    continue
                ot = osp.tile([NCLS, MACRO], mybir.dt.float32, tag="ot")
                nc.vector.tensor_scalar_add(ot[:, :], ops[:, :], fbs[:, :])
                if CFG["batch_store"]:
                    nbc = MACRO // 128
                    ob4 = osp.tile([128, nbc * NCLS], mybir.dt.float32, tag="ob4")
                    for bc in range(nbc):
                        op2 = tpp.tile([128, NCLS], mybir.dt.float32, tag="tp")
                        nc.tensor.transpose(
                            op2[:, :], ot[:, bc * 128 : (bc + 1) * 128], eyfs[:, :]
                        )
                        nc.vector.tensor_copy(
                            ob4[:, bc * NCLS : (bc + 1) * NCLS], op2[:, :]
                        )
                    nc.sync.dma_start(
                        out[m * MACRO : (m + 1) * MACRO, :].rearrange(
                            "(b p) c -> p b c", p=128
                        ),
                        ob4.rearrange("p (b c) -> p b c", c=NCLS),
                    )
                else:
                    for bc in range(MACRO // 128):
                        row0 = m * MACRO + bc * 128
                        op2 = tpp.tile([128, NCLS], mybir.dt.float32, tag="tp")
                        nc.tensor.transpose(
                            op2[:, :], ot[:, bc * 128 : (bc + 1) * 128], eyfs[:, :]
                        )
                        ob = osp.tile([128, NCLS], mybir.dt.float32, tag="ob", name=f"ob_{m}_{bc}")
                        nc.vector.tensor_copy(ob[:, :], op2[:, :])
                        nc.sync.dma_start(out[row0 : row0 + 128, :], ob[:, :])
    if split_waits:
        _split_waits(nc)
    return nc


_CACHED = {}


def _get_nc(bl):
    if bl not in _CACHED:
        _CACHED[bl] = build_nc(bl)
    return _CACHED[bl]


def kernel(x, conv_w, fc_w, fc_b):
    x = np.ascontiguousarray(np.asarray(x, dtype=np.float32))
    conv_w = np.asarray(conv_w, dtype=np.float32)
    fc_w = np.asarray(fc_w, dtype=np.float32)
    fc_b = np.asarray(fc_b, dtype=np.float32)

    wpack, fpack = _host_packs(conv_w, fc_w, fc_b)

    nc = _get_nc(BL)
    in_maps = []
    for c in range(NCORES):
        in_maps.append(
            {
                "x": x[c * BL : (c + 1) * BL],
                "wpack": wpack,
                "fpack": fpack,
            }
        )
    # The axon-proxied NeuronCores occasionally come up wedged
    # (NRT_EXEC_UNIT_UNRECOVERABLE) on the first execute after idle periods;
    # a retry on a fresh execute reliably recovers.
    last_err = None
    for _attempt in range(3):
        try:
            res = run_bass_kernel_spmd(nc, in_maps, core_ids=list(range(NCORES)))
            break
        except Exception as e:  # noqa: BLE001
            last_err = e
            if "UNRECOVERABLE" not in str(e) and "desynced" not in str(e):
                raise
    else:
        raise last_err
    out = np.concatenate([np.asarray(r["out"]) for r in res.results], axis=0)
    return out


if __name__ == "__main__":
    rng = np.random.default_rng(0)
    xs = rng.standard_normal((B, PIX), dtype=np.float32)
    cw = rng.standard_normal((3, 3), dtype=np.float32)
    fw = (rng.standard_normal((NCLS, 676)) * 0.05).astype(np.float32)
    fb = (rng.standard_normal((NCLS,)) * 0.05).astype(np.float32)
    res = kernel(xs, cw, fw, fb)
    print(res.shape, res.dtype)


# revision 56
# speedup vs baseline: 1.1425x; 1.0617x over previous
"""Trainium2 Bass kernel for DigitConvolutionalModel.

Computation: x[B,784] -> reshape [28,28] -> 3x3 valid conv (single channel)
-> relu -> flatten [676] -> linear to 10 classes.

Strategy (pure data parallel over 8 cores, batch-sharded; per core 4096 rows
are processed in 8 macro-tiles of 512):
  * The conv is expressed as a banded matmul: h[b, o] = sum_p x[b, p] * Wband[p, o]
    where o = 28*oi + oj is the conv output embedded in a 768-slot vector
    (stride-28 embedding; invalid o slots have zero weights).  Wband has
    bandwidth 59, so for 128-wide o-chunks only two 128-row pixel-chunk
    blocks of Wband are nonzero -> 2 matmuls per o-chunk (the second with
    K=16 for the last chunk).  All matmul operands are bf16 (fp32 PSUM
    accumulation), rel err vs the fp32 reference ~4e-3.
  * Input loads are SWDGE DMAs with an inline f32->bf16 cast: one DMA per
    macro-tile for the 768 aligned pixel columns, plus a single up-front DMA
    for all 16-column tails.
  * The tensor engine contracts over the partition dim, so x must be
    transposed to pixel-major (x^T).  Most macro-tiles do this with a single
    DMA-xbar transpose instruction ([128, 3072] bf16 -> 24 transposed
    128x128 chunks written straight to SBUF); every pe_every-th macro uses
    TensorE transpose-mode instead, balancing DMA-fabric vs PE occupancy.
    The 16-pixel tails are always PE-transposed.
  * conv matmuls produce h^T [o-chunk, batch] in PSUM (fp32 accumulate);
    ReLU + cast to bf16 happens in the PSUM->SBUF copy on the Scalar
    engine; the FC layer is 6 accumulating matmuls with the (embedded,
    host-prepared) fc weight as the stationary operand, yielding
    out^T [10, batch] in PSUM.  Bias is added via a per-partition
    tensor_scalar during the PSUM->SBUF copy, then small PE transposes
    bring the result back to batch-major and one DMA per macro-tile
    stores [512, 10] contiguously.
  * After Tile scheduling, a post-pass hoists excess per-instruction
    semaphore waits into standalone EventSemaphore instructions (this
    walrus build only accepts one sync-wait command per instruction).
"""

import sys

for _p in ("/opt/trn_rl_repo",):
    if _p not in sys.path:
        sys.path.insert(0, _p)

import ml_dtypes
import numpy as np

import concourse.bass as bass
import concourse.mybir as mybir
from concourse.bass_utils import run_bass_kernel_spmd
from concourse.tile import TileContext

B = 32768
PIX = 784  # 28*28
EMB = 768  # 6*128; embedded conv-output length (o = 28*oi + oj, max 725)
NCLS = 10
NCORES = 8
BL = B // NCORES  # rows per core
MACRO = 512  # batch rows processed per macro-tile
_BF16 = ml_dtypes.bfloat16


def _host_weights(conv_w, fc_w, fc_b):
    """Build the banded conv matrix and embedded fc weights on the host."""
    oi = np.repeat(np.arange(26), 26)
    oj = np.tile(np.arange(26), 26)
    o = 28 * oi + oj  # embedded output index
    wband = np.zeros((896, EMB), np.float32)
    for ki in range(3):
        for kj in range(3):
            wband[o + 28 * ki + kj, o] = conv_w[ki, kj]
    w1 = np.ascontiguousarray(
        np.stack([wband[128 * q : 128 * (q + 1), 128 * q : 128 * (q + 1)] for q in range(6)])
    )
    w2 = np.ascontiguousarray(
        np.stack([wband[128 * (q + 1) : 128 * (q + 2), 128 * q : 128 * (q + 1)] for q in range(6)])
    )
    fct = np.zeros((EMB, NCLS), np.float32)
    fct[o, :] = fc_w[:, 26 * oi + oj].T
    fcw = np.ascontiguousarray(np.stack([fct[128 * q : 128 * (q + 1)] for q in range(6)]))
    return (
        w1.astype(_BF16),
        w2.astype(_BF16),
        fcw.astype(_BF16),
        np.ascontiguousarray(fc_b.reshape(NCLS, 1)).astype(np.float32),
    )


def _host_packs(conv_w, fc_w, fc_b):
    """Pack all constants into two arrays so they load in two DMAs.

    wpack [128, 1724] bf16 = w1 (6x128 cols) | w2 (6x128) | fcw (6x10) | eye128
    fpack [10, 11] f32    = fc_b | eye10
    """
    w1, w2, fcw, fbv = _host_weights(conv_w, fc_w, fc_b)
    cols = [w1[q] for q in range(6)] + [w2[q] for q in range(6)] + [fcw[q] for q in range(6)]
    cols.append(np.eye(128, dtype=np.float32).astype(_BF16))
    wpack = np.ascontiguousarray(np.concatenate(cols, axis=1))
    fpack = np.ascontiguousarray(
        np.concatenate([fbv, np.eye(NCLS, dtype=np.float32)], axis=1)
    ).astype(np.float32)
    return wpack, fpack


def _split_waits(nc, max_waits=1):
    """Hoist excess semaphore waits into standalone EventSemaphore
    instructions.  Walrus codegen only supports a small number of sync-wait
    commands per ISA instruction; the Tile scheduler can attach more
    (producer dep + DMA-lane reuse + first-use deps)."""
    for f in nc.m.functions:
        for blk in f.blocks:
            new = []
            changed = False
            for inst in blk.instructions:
                si = inst.sync_info
                if si is not None and len(si.on_wait) > max_waits:
                    waits = list(si.on_wait)
                    excess, keep = waits[:-max_waits], waits[-max_waits:]
                    for i, w in enumerate(excess):
                        new.append(
                            mybir.InstEventSemaphore(
                                name=f"{inst.name}-prewait{i}",
                                engine=inst.engine,
                                ins=[],
                                outs=[],
                                sync_info=mybir.SyncInfo(on_wait=[w], on_update=[]),
                            )
                        )
                    inst.sync_info = mybir.SyncInfo(
                        on_wait=keep, on_update=list(si.on_update)
                    )
                    changed = True
                new.append(inst)
            if changed:
                blk.instructions = new


# Tunables (A/B tested against the CoreSim cost model).
CFG = {
    "load_mode": "swdge_cast",  # or "hwdge_f32"
    "xin_bufs": 4,
    "xt_bufs": 2,
    "ht_bufs": 3,
    "tp_bufs": 4,
    "hp_bufs": 3,
    "op_bufs": 1,
    "batch_store": True,
    "stage": 4,  # ablation: 1=loads 2=+transposes 3=+conv/relu 4=full
    "xbar": True,  # transpose x chunks via DMA xbar instead of TensorE
    "fused": True,  # one load + one xbar transpose per macro
    "pe_every": 3,  # in fused mode, every Nth macro transposes on TensorE
    "pe_set": [0],  # explicit macro indices using TensorE transposes (overrides pe_every)
    "load_mult": 1,  # macros per input-load DMA
    "relu_dve": 1,  # how many of the 6 per-macro relus run on DVE (rest ACT)
    "prefetch": 3,  # macro loads emitted before the constants at kernel start
}


def build_nc(bl=BL, split_waits=True):
    # Default SWDGE descriptor ring (16 KiB = 1024 descs) only holds ~2 input
    # loads' descriptors, serializing descriptor-gen behind transfers.
    nc = bass.Bass("TRN2", dynamic_dma_scratch_size=CFG.get("dma_scratch", 65536))
    x = nc.dram_tensor("x", [bl, PIX], mybir.dt.float32, kind="ExternalInput")
    wpk = nc.dram_tensor("wpack", [128, 1724], mybir.dt.bfloat16, kind="ExternalInput")
    fpk = nc.dram_tensor("fpack", [NCLS, 11], mybir.dt.float32, kind="ExternalInput")
    out = nc.dram_tensor("out", [bl, NCLS], mybir.dt.float32, kind="ExternalOutput")

    nm = bl // MACRO
    relu = mybir.ActivationFunctionType.Relu

    with TileContext(nc) as tc:
        with (
            tc.tile_pool(name="const", bufs=1) as cp,
            tc.tile_pool(name="xin", bufs=CFG["xin_bufs"]) as xp,
            tc.tile_pool(name="xt", bufs=CFG["xt_bufs"]) as xtp,
            tc.tile_pool(name="ht", bufs=CFG["ht_bufs"]) as htp,
            tc.tile_pool(name="osb", bufs=3) as osp,
            tc.tile_pool(name="tps", bufs=CFG["tp_bufs"], space="PSUM") as tpp,
            tc.tile_pool(name="hps", bufs=CFG["hp_bufs"], space="PSUM") as hpp,
            tc.tile_pool(name="ops", bufs=CFG["op_bufs"], space="PSUM") as opp,
        ):
            nbc = MACRO // 128
            _xfl_cache = [None]
            _prefetched = {}
            # Emit the first macro loads BEFORE the constants so the DMA
            # engines start streaming x immediately (PE startup latency), and
            # so the serialized load->xbar chain starts with a full pipeline.
            if CFG["fused"]:
                for pm in range(min(CFG.get("prefetch", 1), bl // MACRO)):
                    xflp = xp.tile(
                        [128, nbc, 768], mybir.dt.bfloat16, tag="xf4", name=f"xf4_pre{pm}"
                    )
                    nc.gpsimd.dma_start(
                        out=xflp[:, :, :],
                        in_=x[pm * MACRO : (pm + 1) * MACRO, 0:768].rearrange(
                            "(g p) c -> p g c", p=128
                        ),
                    )
                    _prefetched[pm] = xflp

            wps = cp.tile([128, 1724], mybir.dt.bfloat16)
            nc.sync.dma_start(wps[:, :], wpk[:, :])
            fps = cp.tile([NCLS, 11], mybir.dt.float32)
            nc.sync.dma_start(fps[:, :], fpk[:, :])
            w1s = wps[:, 0:768]
            w2s = wps[:, 768:1536]
            fcs = wps[:, 1536:1596]
            eybs = wps[:, 1596:1724]
            fbs = fps[:, 0:1]
            eyfs = fps[:, 1:11]

            xtl_all = None
            if CFG["fused"]:
                # All pixel-tail columns (768:784) for the whole shard in one
                # DMA -- per-macro tail loads would each pay the DMA fixed cost.
                xtl_all = cp.tile([128, bl // 128, 16], mybir.dt.bfloat16)
                nc.gpsimd.dma_start(
                    out=xtl_all[:, :, :],
                    in_=x[:, 768:784].rearrange("(g p) c -> p g c", p=128),
                )

            for m in range(nm):
                if CFG["fused"]:
                    lm = CFG["load_mult"]
                    if m in _prefetched:
                        _xfl_cache[0] = _prefetched[m]
                    elif m % lm == 0:
                        # [p, g, col]: batch row 128g+p, pixel col (768 aligned)
                        xfl = xp.tile(
                            [128, lm * nbc, 768], mybir.dt.bfloat16, tag="xf4"
                        )
                        r0 = m * MACRO
                        ld = nc.gpsimd.dma_start(
                            out=xfl[:, :, :],
                            in_=x[r0 : r0 + lm * MACRO, 0:768].rearrange(
                                "(g p) c -> p g c", p=128
                            ),
                        )
                        if CFG.get("dma_prio"):
                            ld.ins.bass_priority = -10000 + m
                        _xfl_cache[0] = xfl
                    xf4 = _xfl_cache[0][:, (m % lm) * nbc : (m % lm + 1) * nbc, :]
                    if CFG["stage"] < 2:
                        continue
                    # xtf[p, 6g + c, b] = xf4[b, g, 128c + p]; most macros use a
                    # single xbar-transpose DMA, every pe_every-th macro uses
                    # TensorE transposes instead to balance the DMA vs PE load.
                    xtf = xtp.tile([128, nbc * 6, 128], mybir.dt.bfloat16, tag="xtf")
                    if CFG.get("pe_set") is not None:
                        use_pe = m in CFG["pe_set"]
                    else:
                        use_pe = CFG["pe_every"] and m % CFG["pe_every"] == 0
                    if use_pe:
                        for g in range(nbc):
                            for c in range(6):
                                tp = tpp.tile([128, 128], mybir.dt.bfloat16, tag="tp")
                                nc.tensor.transpose(
                                    tp[:, :],
                                    xf4[:, g, 128 * c : 128 * (c + 1)],
                                    eybs[:, :],
                                )
                                dst = xtf[:, 6 * g + c, :]
                                if c % 2 == 0:
                                    nc.vector.tensor_copy(dst, tp[:, :])
                                else:
                                    nc.scalar.copy(dst, tp[:, :])
                    else:
                        xb = nc.sync.dma_start_transpose(xtf[:, :, :], xf4[:, :, :])
                        if CFG.get("dma_prio"):
                            xb.ins.bass_priority = -9000 + m
                    # conv rhs view: chunk c -> [p, (g, b)]
                    xt = xtf.rearrange("p (g c2) b -> p c2 g b", g=nbc)
                    xt6 = xtp.tile([16, MACRO], mybir.dt.bfloat16, tag="xt6")
                    for g in range(nbc):
                        tp6 = tpp.tile([16, 128], mybir.dt.bfloat16, tag="tp")
                        nc.tensor.transpose(tp6[:, :], xtl_all[:, m * nbc + g, :], eybs[:, :])
                        dst6 = xt6[:, g * 128 : (g + 1) * 128]
                        if g % 2 == 0:
                            nc.scalar.copy(dst6, tp6[:, :])
                        else:
                            nc.vector.tensor_copy(dst6, tp6[:, :])
                    bc_iter = []
                else:
                    xt = xtp.tile([128, 6, MACRO], mybir.dt.bfloat16, tag="xt")
                    xt6 = xtp.tile([16, MACRO], mybir.dt.bfloat16, tag="xt6")
                    bc_iter = range(MACRO // 128)
                for bc in bc_iter:
                    row0 = m * MACRO + bc * 128
                    xf = xp.tile([128, PIX], mybir.dt.bfloat16, tag="xf", name=f"xf_{m}_{bc}")
                    if CFG["load_mode"] == "swdge_cast":
                        # SWDGE DMA with inline f32 -> bf16 cast
                        nc.gpsimd.dma_start(out=xf[:, :], in_=x[row0 : row0 + 128, :])
                    else:
                        xf32 = xp.tile(
                            [128, PIX], mybir.dt.float32, tag="xf32", name=f"xf32_{m}_{bc}"
                        )
                        nc.sync.dma_start(out=xf32[:, :], in_=x[row0 : row0 + 128, :])
                        nc.gpsimd.tensor_copy(xf[:, :], xf32[:, :])
                    if CFG["stage"] < 2:
                        continue
                    if CFG["xbar"]:
                        # One xbar-transpose DMA moves all 6 full 128x128
                        # chunks: out[p, c, b] = xf[b, 128c + p].
                        nc.sync.dma_start_transpose(
                            xt[:, :, bc * 128 : (bc + 1) * 128], xf[:, 0:768]
                        )
                    else:
                        for c in range(6):
                            tp = tpp.tile([128, 128], mybir.dt.bfloat16, tag="tp")
                            nc.tensor.transpose(
                                tp[:, :], xf[:, 128 * c : 128 * (c + 1)], eybs[:, :]
                            )
                            dst = xt[:, c, bc * 128 : (bc + 1) * 128]
                            if c % 2 == 0:
                                nc.vector.tensor_copy(dst, tp[:, :])
                            else:
                                nc.scalar.copy(dst, tp[:, :])
                    tp6 = tpp.tile([16, 128], mybir.dt.bfloat16, tag="tp")
                    nc.tensor.transpose(tp6[:, :], xf[:, 768:784], eybs[:, :])
                    nc.vector.tensor_copy(xt6[:, bc * 128 : (bc + 1) * 128], tp6[:, :])

                if CFG["stage"] < 3:
                    continue
                ops = opp.tile([NCLS, MACRO], mybir.dt.float32, tag="ops")
                for q in range(6):
                    hp = hpp.tile([128, MACRO], mybir.dt.float32, tag="hp")
                    nc.tensor.matmul(
                        hp[:, :], w1s[:, 128 * q : 128 * (q + 1)], xt[:, q],
                        start=True, stop=False,
                    )
                    if q < 5:
                        nc.tensor.matmul(
                            hp[:, :], w2s[:, 128 * q : 128 * (q + 1)], xt[:, q + 1],
                            start=False, stop=True,
                        )
                    else:
                        nc.tensor.matmul(
                            hp[:, :], w2s[0:16, 128 * 5 : 128 * 6], xt6[:, :],
                            start=False, stop=True,
                        )
                    ht = htp.tile([128, MACRO], mybir.dt.bfloat16, tag="ht")
                    # ReLU during PSUM->SBUF drain; split between the Scalar
                    # and Vector engines to balance their load.
                    if q < CFG["relu_dve"]:
                        nc.vector.tensor_scalar_max(ht[:, :], hp[:, :], 0.0)
                    else:
                        nc.scalar.activation(ht[:, :], hp[:, :], relu)
                    if CFG["stage"] >= 4:
                        nc.tensor.matmul(
                            ops[:, :], fcs[:, NCLS * q : NCLS * (q + 1)], ht[:, :],
                            start=(q == 0), stop=(q == 5),
                        )

                if CFG["stage"] < 4:
                    continue
                ot = osp.tile([NCLS, MACRO], mybir.dt.float32, tag="ot")
                nc.vector.tensor_scalar_add(ot[:, :], ops[:, :], fbs[:, :])
                if CFG["batch_store"]:
                    nbc = MACRO // 128
                    ob4 = osp.tile([128, nbc * NCLS], mybir.dt.float32, tag="ob4")
                    for bc in range(nbc):
                        op2 = tpp.tile([128, NCLS], mybir.dt.float32, tag="tp")
                        nc.tensor.transpose(
                            op2[:, :], ot[:, bc * 128 : (bc + 1) * 128], eyfs[:, :]
                        )
                        nc.vector.tensor_copy(
                            ob4[:, bc * NCLS : (bc + 1) * NCLS], op2[:, :]
                        )
                    nc.sync.dma_start(
                        out[m * MACRO : (m + 1) * MACRO, :].rearrange(
                            "(b p) c -> p b c", p=128
                        ),
                        ob4.rearrange("p (b c) -> p b c", c=NCLS),
                    )
                else:
                    for bc in range(MACRO // 128):
                        row0 = m * MACRO + bc * 128
                        op2 = tpp.tile([128, NCLS], mybir.dt.float32, tag="tp")
                        nc.tensor.transpose(
                            op2[:, :], ot[:, bc * 128 : (bc + 1) * 128], eyfs[:, :]
                        )
                        ob = osp.tile([128, NCLS], mybir.dt.float32, tag="ob", name=f"ob_{m}_{bc}")
                        nc.vector.tensor_copy(ob[:, :], op2[:, :])
                        nc.sync.dma_start(out[row0 : row0 + 128, :], ob[:, :])
    if split_waits:
        _split_waits(nc)
    return nc


_CACHED = {}


def _get_nc(bl):
    if bl not in _CACHED:
        _CACHED[bl] = build_nc(bl)
    return _CACHED[bl]


def kernel(x, conv_w, fc_w, fc_b):
    x = np.ascontiguousarray(np.asarray(x, dtype=np.float32))
    conv_w = np.asarray(conv_w, dtype=np.float32)
    fc_w = np.asarray(fc_w, dtype=np.float32)
    fc_b = np.asarray(fc_b, dtype=np.float32)

    wpack, fpack = _host_packs(conv_w, fc_w, fc_b)

    nc = _get_nc(BL)
    in_maps = []
    for c in range(NCORES):
        in_maps.append(
            {
                "x": x[c * BL : (c + 1) * BL],
                "wpack": wpack,
                "fpack": fpack,
            }
        )
    # The axon-proxied NeuronCores occasionally come up wedged
    # (NRT_EXEC_UNIT_UNRECOVERABLE) on the first execute after idle periods;
    # a retry on a fresh execute reliably recovers.
    last_err = None
    for _attempt in range(3):
        try:
            res = run_bass_kernel_spmd(nc, in_maps, core_ids=list(range(NCORES)))
            break
        except Exception as e:  # noqa: BLE001
            last_err = e
            if "UNRECOVERABLE" not in str(e) and "desynced" not in str(e):
                raise
    else:
        raise last_err
    out = np.concatenate([np.asarray(r["out"]) for r in res.results], axis=0)
    return out


if __name__ == "__main__":
    rng = np.random.default_rng(0)
    xs = rng.standard_normal((B, PIX), dtype=np.float32)
    cw = rng.standard_normal((3, 3), dtype=np.float32)
    fw = (rng.standard_normal((NCLS, 676)) * 0.05).astype(np.float32)
    fb = (rng.standard_normal((NCLS,)) * 0.05).astype(np.float32)
    res = kernel(xs, cw, fw, fb)
    print(res.shape, res.dtype)
